# revision 1
# baseline (speedup 1.0000x reference)
"""MoE downsample kernel for 8 TRN2 NeuronCores.

Strategy: data-parallel over batch (2 samples per core). Each strided dilated
conv is decomposed into k*k "tap" matmuls (lhsT = [Cin=64, Cout=64] weight
slice, rhs = strided view of the zero-padded input image) accumulated in PSUM
over 512-pixel output chunks. Samples map to PE-array row halves (partitions
0-63 / 64-127) and two balanced expert queues map to PE col halves, so four
64x64 tile_position matmuls run concurrently (full 128x128 array).
BN + conv-bias + GELU are fused into the ScalarE PSUM eviction.
Gating (tiny: 16x64x4 matvec + softmax + top2) and final top-2 assembly run
on host.
"""

import numpy as np
import ml_dtypes

KS = [3, 5, 7, 9]
DS = [1, 2, 3, 4]
BN_EPS = 1e-5
B, CIN, H, W = 16, 64, 256, 256
CE = 64
PAD = 16          # left/top pad (max |offset|); right/bottom needs 15
HP = WP = PAD + 256 + 15   # 287
HO = WO = 128
NCORES = 8
SPC = 2           # samples per core
NTAPS = sum(k * k for k in KS)  # 164
CHUNK_ROWS = 4    # output rows per 512-px chunk
NCHUNKS = HO // CHUNK_ROWS      # 32

# tap slot base per expert
_SLOT_BASE = np.cumsum([0] + [k * k for k in KS]).tolist()

# queue split: col0 = experts [0,1,2] (83 taps), col1 = [3] (81 taps)
COL_EXPERTS = [[0, 1, 2], [3]]

_COMPILED = None


def _tap_offsets(e):
    """Yield (slot, row_off, col_off) in padded coords for expert e, tap (u,v)."""
    k, d = KS[e], DS[e]
    pad = d * (k - 1) // 2
    for u in range(k):
        for v in range(k):
            slot = _SLOT_BASE[e] + u * k + v
            yield slot, d * u - pad + PAD, d * v - pad + PAD


def _build_program():
    import concourse.bass as bass  # noqa: F401
    import concourse.mybir as mybir
    import concourse.tile as tile
    from concourse import bacc
    from contextlib import ExitStack

    dt = mybir.dt
    nc = bacc.Bacc("TRN2", target_bir_lowering=False, debug=False,
                   num_devices=NCORES)
    xpad = nc.dram_tensor("xpad", [SPC, CIN, HP, WP], dt.bfloat16,
                          kind="ExternalInput")
    wt = nc.dram_tensor("wt", [CIN, NTAPS, CE], dt.bfloat16,
                        kind="ExternalInput")
    bnp = nc.dram_tensor("bnp", [CE, 4, 2], dt.float32, kind="ExternalInput")
    out = nc.dram_tensor("out", [SPC, 4, CE, HO, WO], dt.float32,
                         kind="ExternalOutput")

    with tile.TileContext(nc) as tc:
        with ExitStack() as ctx:
            consts = ctx.enter_context(tc.tile_pool(name="consts", bufs=1))
            stage_pool = ctx.enter_context(tc.tile_pool(name="st", bufs=8))

            # ---- load constants / inputs into SBUF ----
            # first x stripe (rows needed by chunk 0) goes out first so the
            # PE can start as early as possible; weights ride alongside.
            wtile = consts.tile([128, NTAPS, CE], dt.bfloat16)
            bntile = consts.tile([128, 4, 2], dt.float32)
            xtile = consts.tile([128, HP, WP], dt.bfloat16)
            bounds = [0, 44] + [44 + ((HP - 44) * p) // 14
                                for p in range(1, 15)]
            for s in range(SPC):
                nc.gpsimd.dma_start(
                    out=xtile[s * 64:(s + 1) * 64, 0:44, :],
                    in_=xpad[s, :, 0:44, :])
            for half in range(2):
                p0 = half * 64
                nc.gpsimd.dma_start(out=wtile[p0:p0 + 64, :, :], in_=wt.ap())
                nc.gpsimd.dma_start(out=bntile[p0:p0 + 64, :, :], in_=bnp.ap())
            for piece in range(1, len(bounds) - 1):
                for s in range(SPC):
                    r0, r1 = bounds[piece], bounds[piece + 1]
                    nc.gpsimd.dma_start(
                        out=xtile[s * 64:(s + 1) * 64, r0:r1, :],
                        in_=xpad[s, :, r0:r1, :])

            psum_pool = ctx.enter_context(
                tc.tile_pool(name="ps", bufs=8, space="PSUM"))

            # ---- main loop: 32 chunks x (2 samples x 2 col-queues) ----
            def queue_events(s, col, r):
                p0 = s * 64           # rhs/lhsT partition base (PE rows)
                q0 = col * 64         # psum/out partition base (PE cols)
                i0 = r * CHUNK_ROWS   # first output row
                for e in COL_EXPERTS[col]:
                    ps = psum_pool.tile([128, 512], dt.float32)
                    taps = list(_tap_offsets(e))
                    for t, (slot, ro, co) in enumerate(taps):
                        r_lo = 2 * i0 + ro
                        rhs = xtile[p0:p0 + 64,
                                    r_lo:r_lo + 2 * CHUNK_ROWS - 1:2,
                                    co:co + 2 * WO - 1:2]
                        lhsT = wtile[p0:p0 + 64, slot, :]
                        psv = ps[q0:q0 + 64, :]
                        first = t == 0
                        last = t == len(taps) - 1

                        def mm(rhs=rhs, lhsT=lhsT, psv=psv, first=first,
                               last=last, p0=p0, q0=q0):
                            nc.tensor.matmul(psv, lhsT, rhs, start=first,
                                             stop=last,
                                             tile_position=(p0, q0))
                        yield ("mm", mm)

                    def evict(ps=ps, s=s, e=e, i0=i0, q0=q0):
                        st = stage_pool.tile([128, CHUNK_ROWS, WO],
                                             dt.float32)
                        nc.scalar.activation(
                            st[q0:q0 + 64, :, :],
                            ps[q0:q0 + 64, :].rearrange(
                                "p (a b) -> p a b", a=CHUNK_ROWS),
                            mybir.ActivationFunctionType.Gelu,
                            scale=bntile[q0:q0 + 64, e, 0:1],
                            bias=bntile[q0:q0 + 64, e, 1:2])
                        nc.sync.dma_start(
                            out=out[s, e, :, i0:i0 + CHUNK_ROWS, :],
                            in_=st[q0:q0 + 64, :, :])
                    yield ("evict", evict)

            for r in range(NCHUNKS):
                queues = [queue_events(s, col, r)
                          for s in range(SPC) for col in range(2)]
                live = list(queues)
                while live:
                    nxt = []
                    for q in live:
                        ev = next(q, None)
                        if ev is None:
                            continue
                        ev[1]()
                        nxt.append(q)
                    live = nxt

    nc.compile()
    return nc


def _get_program():
    global _COMPILED
    if _COMPILED is None:
        _COMPILED = _build_program()
    return _COMPILED


def _host_gate(x, gate_w, gate_b):
    """Replicate reference gating in numpy (f64 pooling for robustness)."""
    pooled = x.astype(np.float64).mean(axis=(2, 3)).astype(np.float32)
    logits = pooled @ gate_w.T.astype(np.float32) + gate_b
    z = logits - logits.max(axis=1, keepdims=True)
    ez = np.exp(z.astype(np.float32))
    gates = ez / ez.sum(axis=1, keepdims=True)
    idx = np.argsort(-gates, axis=1, kind="stable")[:, :2]
    wsel = np.take_along_axis(gates, idx, axis=1)
    wsel = wsel / (wsel.sum(axis=1, keepdims=True) + 1e-8)
    return idx, wsel.astype(np.float32)


def _prep_inputs(x, ws, bs, bn_scale, bn_bias, bn_mean, bn_var):
    bf16 = ml_dtypes.bfloat16
    # padded bf16 images, per core
    xpad = np.zeros((B, CIN, HP, WP), dtype=bf16)
    xpad[:, :, PAD:PAD + H, PAD:PAD + W] = x.astype(bf16)

    # transposed weights, DMA-friendly layout [CIN, NTAPS, CE]
    wt = np.empty((CIN, NTAPS, CE), dtype=bf16)
    for e in range(4):
        k = KS[e]
        w = ws[e].astype(np.float32)  # [CE, CIN, k, k]
        # [CE, CIN, k, k] -> [CIN, k*k, CE]
        wt[:, _SLOT_BASE[e]:_SLOT_BASE[e] + k * k, :] = (
            w.transpose(1, 2, 3, 0).reshape(CIN, k * k, CE).astype(bf16))

    # folded BN: z = conv*scale + shift ; scale = bn_scale*rsqrt(var+eps),
    # shift = conv_bias*scale + bn_bias - mean*scale
    inv = (bn_scale / np.sqrt(bn_var + BN_EPS)).astype(np.float32)
    shift = (np.stack(bs) * inv + bn_bias - bn_mean * inv).astype(np.float32)
    bnp = np.stack([inv, shift], axis=1)  # [4, 2, CE]
    bnp = np.ascontiguousarray(bnp.transpose(2, 0, 1))  # [CE, 4, 2]
    return xpad, wt, bnp


def run(inputs, trace=False):
    from concourse import bass_utils

    x = np.asarray(inputs["x"], dtype=np.float32)
    ws = [np.asarray(inputs[f"w{i}"], dtype=np.float32) for i in range(4)]
    bs = [np.asarray(inputs[f"b{i}"], dtype=np.float32) for i in range(4)]
    bn_scale = np.asarray(inputs["bn_scale"], dtype=np.float32)
    bn_bias = np.asarray(inputs["bn_bias"], dtype=np.float32)
    bn_mean = np.asarray(inputs["bn_mean"], dtype=np.float32)
    bn_var = np.asarray(inputs["bn_var"], dtype=np.float32)
    gate_w = np.asarray(inputs["gate_w"], dtype=np.float32)
    gate_b = np.asarray(inputs["gate_b"], dtype=np.float32)

    nc = _get_program()
    xpad, wt, bnp = _prep_inputs(x, ws, bs, bn_scale, bn_bias, bn_mean,
                                 bn_var)
    in_maps = []
    for c in range(NCORES):
        in_maps.append({
            "xpad": xpad[c * SPC:(c + 1) * SPC],
            "wt": wt,
            "bnp": bnp,
        })
    res = bass_utils.run_bass_kernel_spmd(
        nc, in_maps, core_ids=list(range(NCORES)), trace=trace)

    # assemble: E[b, e] for all experts, then host top-2 select/scale/concat
    E = np.concatenate([res.results[c]["out"] for c in range(NCORES)],
                       axis=0)  # [B, 4, CE, HO, WO]
    idx, wsel = _host_gate(x, gate_w, gate_b)
    outf = np.empty((B, 2 * CE, HO, WO), dtype=np.float32)
    for b in range(B):
        outf[b, :CE] = E[b, idx[b, 0]] * wsel[b, 0]
        outf[b, CE:] = E[b, idx[b, 1]] * wsel[b, 1]
    return outf, res


def kernel(**inputs):
    outf, _ = run(inputs, trace=False)
    return outf

